# revision 5
# baseline (speedup 1.0000x reference)
"""GCNConv Trainium2 kernel: out = D^{-1/2} A D^{-1/2} (X @ W).

Strategy (8 NeuronCores, 1D row partition of the uniform-degree CSR):
  - each core owns 12500 destination nodes (padded to 12544 = 98*128)
  - phase A: X_k @ W, row-scaled by d_j  ->  X'' shard in bf16
  - phase B: AllGather bf16 shards -> full [100352, 64] bf16 table per core
  - phase C: per 128-node tile, indirect DMA gathers the 16 neighbor
    rows of each node (packed 4/elem, 512B), DVE mask-select + reduce,
    scale by d_i, store f32.
Experiment split: tiles < SPLIT use one 2048-idx gather call; tiles >=
SPLIT use two 1024-idx calls (to measure SWDGE prep scaling).
"""

import numpy as np

N_NODES = 100000
D_IN = 256
D_OUT = 64
DEG = 16
N_CORES = 8
P = 128
SHARD = N_NODES // N_CORES            # 12500
N_TILES = (SHARD + P - 1) // P        # 98
NPAD = N_TILES * P                    # 12544
SPLIT = 49                            # tiles<SPLIT: 2x1024-idx calls; rest: 4x512

_CACHE = {}


def _build_program(n_tiles=N_TILES, deg=DEG, d_in=D_IN, d_out=D_OUT,
                   n_cores=N_CORES):
    import concourse.bacc as bacc
    from concourse import bass, mybir, tile
    from concourse.masks import make_identity

    npad = n_tiles * P
    f32 = mybir.dt.float32
    bf16 = mybir.dt.bfloat16

    nc = bacc.Bacc("TRN2", target_bir_lowering=False, debug=False,
                   num_devices=n_cores)
    Xs = nc.dram_tensor("Xs", [npad, d_in], f32, kind="ExternalInput").ap()
    W = nc.dram_tensor("W", [d_in, d_out], f32, kind="ExternalInput").ap()
    degs = nc.dram_tensor("degs", [npad, 1], f32, kind="ExternalInput").ap()
    # gather inputs: node//4 indices (int16), wrapped layouts for both call
    # shapes + residue one-hot masks (bf16)
    idxA = nc.dram_tensor("idxA", [n_tiles, P, deg * P // 16],
                          mybir.dt.int16, kind="ExternalInput").ap()
    idxB = nc.dram_tensor("idxB", [n_tiles, 2, P, (deg // 2) * P // 16],
                          mybir.dt.int16, kind="ExternalInput").ap()
    msks = nc.dram_tensor("msks", [n_tiles, P, 4 * deg], bf16,
                          kind="ExternalInput").ap()
    out = nc.dram_tensor("out", [npad, d_out], f32, kind="ExternalOutput").ap()

    n_kchunk = d_in // P  # 2

    with tile.TileContext(nc) as tc:
        with (
            tc.tile_pool(name="const", bufs=1) as constp,
            tc.tile_pool(name="xin", bufs=3) as xinp,
            tc.tile_pool(name="xtr", bufs=3) as xtp,
            tc.tile_pool(name="ps", bufs=2, space="PSUM") as psp,
            tc.tile_pool(name="xp", bufs=3) as xpp,
            tc.tile_pool(name="dg", bufs=3) as degp,
            tc.tile_pool(name="ix", bufs=8) as idxp,
            tc.tile_pool(name="gt", bufs=3) as gp,
            tc.tile_pool(name="ot", bufs=3) as outp,
            tc.tile_pool(name="dram", bufs=1, space="DRAM") as dramp,
        ):
            identity = constp.tile([P, P], f32)
            make_identity(nc, identity[:])
            w_sb = constp.tile([P, n_kchunk * d_out], f32)
            for c in range(n_kchunk):
                nc.sync.dma_start(out=w_sb[:, c * d_out:(c + 1) * d_out],
                                  in_=W[c * P:(c + 1) * P, :])

            xpd = dramp.tile([npad, d_out], bf16)
            xfull = dramp.tile([n_cores * npad, d_out], bf16,
                               addr_space="Shared")

            # ---- Phase A: X'' = (X @ W) * d, cast to bf16 ----
            for t in range(n_tiles):
                sl = slice(t * P, (t + 1) * P)
                xt_t = xinp.tile([P, d_in], f32)
                nc.sync.dma_start(out=xt_t[:], in_=Xs[sl, :])
                deg_t = degp.tile([P, 1], f32)
                nc.sync.dma_start(out=deg_t[:], in_=degs[sl, :])
                pso = psp.tile([P, d_out], f32, space="PSUM")
                for c in range(n_kchunk):
                    psT = psp.tile([P, P], f32, space="PSUM", tag="psT")
                    nc.tensor.transpose(psT[:], xt_t[:, c * P:(c + 1) * P],
                                        identity[:])
                    xT = xtp.tile([P, P], f32, tag="xT")
                    if c % 2 == 0:
                        nc.scalar.copy(xT[:], psT[:])
                    else:
                        nc.vector.tensor_copy(xT[:], psT[:])
                    nc.tensor.matmul(pso[:], xT[:],
                                     w_sb[:, c * d_out:(c + 1) * d_out],
                                     start=(c == 0), stop=(c == n_kchunk - 1))
                xp_t = xpp.tile([P, d_out], bf16)
                nc.vector.tensor_scalar_mul(xp_t[:], pso[:], deg_t[:, 0:1])
                nc.sync.dma_start(out=xpd[sl, :], in_=xp_t[:])

            # ---- Phase B: AllGather shards (bf16) ----
            nc.gpsimd.collective_compute(
                "AllGather", mybir.AluOpType.bypass,
                replica_groups=[list(range(n_cores))],
                ins=[xpd.opt()], outs=[xfull.opt()],
            )

            # ---- Phase C: bulk dma_gather on the 4-packed bf16 table view
            # (idx = node//4 fits int16), residue-mask select + reduce ----
            xpk = xfull[:].rearrange("(a b) f -> a (b f)", b=4)  # [N/4, 256]
            pk = 4 * d_out  # 256 bf16 elems = 512B per packed row
            half = deg // 2
            for t in range(n_tiles):
                sl = slice(t * P, (t + 1) * P)
                msk_t = idxp.tile([P, 4 * deg], bf16, tag="msk")
                nc.sync.dma_start(out=msk_t[:], in_=msks[t])
                deg_c = degp.tile([P, 1], f32, tag="deg_c")
                nc.sync.dma_start(out=deg_c[:], in_=degs[sl, :])
                g_t = gp.tile([P, deg * pk], bf16, tag="g")
                if t < SPLIT:
                    # two 1024-idx calls (idxB layout halves)
                    for h in range(2):
                        idx_t = idxp.tile([P, half * P // 16], mybir.dt.int16,
                                          tag="idxb")
                        nc.sync.dma_start(out=idx_t[:], in_=idxB[t, h])
                        nc.gpsimd.dma_gather(
                            g_t[:, h * half * pk:(h + 1) * half * pk]
                            .rearrange("p (s f) -> p s f", s=half),
                            xpk, idx_t[:], half * P, half * P, pk)
                else:
                    # four 512-idx calls (quarters of the idxA layout)
                    qtr = deg // 4
                    idx_t = idxp.tile([P, deg * P // 16], mybir.dt.int16,
                                      tag="idxa")
                    nc.sync.dma_start(out=idx_t[:], in_=idxA[t])
                    for h in range(4):
                        nc.gpsimd.dma_gather(
                            g_t[:, h * qtr * pk:(h + 1) * qtr * pk]
                            .rearrange("p (s f) -> p s f", s=qtr),
                            xpk, idx_t[:, h * qtr * P // 16:
                                       (h + 1) * qtr * P // 16],
                            qtr * P, qtr * P, pk)
                prod = gp.tile([P, deg * pk], bf16, tag="prod")
                nc.vector.tensor_tensor(
                    out=prod[:].rearrange("p (s q f) -> p s q f",
                                          s=deg, q=4),
                    in0=g_t[:].rearrange("p (s q f) -> p s q f",
                                         s=deg, q=4),
                    in1=msk_t[:].rearrange("p (s q) -> p s q", q=4)
                    .to_broadcast([P, deg, 4, d_out]),
                    op=mybir.AluOpType.mult)
                r_t = outp.tile([P, d_out], f32, tag="r")
                nc.vector.tensor_reduce(
                    r_t[:],
                    prod[:].rearrange("p (s q f) -> p f s q", s=deg, q=4),
                    axis=mybir.AxisListType.XY, op=mybir.AluOpType.add)
                o_t = outp.tile([P, d_out], f32, tag="o_t")
                nc.vector.tensor_scalar_mul(o_t[:], r_t[:], deg_c[:, 0:1])
                nc.sync.dma_start(out=out[sl, :], in_=o_t[:])

    nc.compile()
    return nc


def _get_program():
    key = "main"
    if key not in _CACHE:
        _CACHE[key] = _build_program()
    return _CACHE[key]


def _prep_inputs(X, weights, column_index, degrees,
                 n_nodes=N_NODES, n_cores=N_CORES, shard=SHARD, npad=NPAD,
                 deg=DEG):
    """Shard + pad host arrays; remap columns to padded AllGather layout."""
    X = np.ascontiguousarray(np.asarray(X, dtype=np.float32))
    W = np.ascontiguousarray(np.asarray(weights, dtype=np.float32))
    col = np.asarray(column_index).astype(np.int64, copy=False)
    dg = np.asarray(degrees, dtype=np.float32)

    # remap node id -> row in the AllGather-concatenated padded table
    col32 = (col // shard * npad + col % shard).astype(np.int32)
    col32 = col32.reshape(n_cores, shard, deg)

    n_tiles = npad // 128
    half = deg // 2
    in_maps = []
    pad = npad - shard
    iwA = np.arange(deg * 128)
    iwB = np.arange(half * 128)
    bf16 = np.dtype('bfloat16') if hasattr(np, 'bfloat16') else None
    import ml_dtypes
    bf16 = ml_dtypes.bfloat16
    for c in range(n_cores):
        Xc = np.concatenate(
            [X[c * shard:(c + 1) * shard],
             np.zeros((pad, X.shape[1]), np.float32)], axis=0)
        dgc = np.concatenate(
            [dg[c * shard:(c + 1) * shard],
             np.zeros(pad, np.float32)], axis=0).reshape(npad, 1)
        ixc = np.concatenate(
            [col32[c], np.zeros((pad, deg), np.int32)], axis=0)
        q4, r4 = ixc // 4, ixc % 4                      # [npad, deg]
        # idxA[t]: single-call wrapped layout (2048 idx); idxB[t,h]: two
        # 1024-idx calls. logical i -> wrap[i%16, i//16], tiled to 128 rows
        idxA = np.zeros((n_tiles, 128, deg * 128 // 16), np.int16)
        idxB = np.zeros((n_tiles, 2, 128, half * 128 // 16), np.int16)
        for t in range(n_tiles):
            blk = q4[t * 128:(t + 1) * 128]             # [128, deg]
            arrA = blk.T.reshape(-1)                    # i = s*128 + p
            wrapA = np.zeros((16, deg * 128 // 16), np.int16)
            wrapA[iwA % 16, iwA // 16] = arrA
            idxA[t] = np.tile(wrapA, (8, 1))
            for h in range(2):
                arr = blk[:, h * half:(h + 1) * half].T.reshape(-1)
                wrap = np.zeros((16, half * 128 // 16), np.int16)
                wrap[iwB % 16, iwB // 16] = arr
                idxB[t, h] = np.tile(wrap, (8, 1))
        # msk[t, p, s*4+q] = 1.0 where residue matches
        msk = (r4[:, :, None] == np.arange(4)[None, None, :]).astype(
            np.float32).reshape(n_tiles, 128, deg * 4).astype(bf16)
        in_maps.append({"Xs": Xc, "W": W, "degs": dgc,
                        "idxA": idxA, "idxB": idxB, "msks": msk})
    return in_maps


def kernel(X, weights, row_pointers, column_index, degrees):
    from concourse.bass_utils import run_bass_kernel_spmd

    rp = np.asarray(row_pointers)
    assert rp.shape[0] == N_NODES + 1
    in_maps = _prep_inputs(X, weights, column_index, degrees)
    nc = _get_program()
    res = run_bass_kernel_spmd(nc, in_maps, core_ids=list(range(N_CORES)))
    outs = [res.results[c]["out"][:SHARD] for c in range(N_CORES)]
    return np.concatenate(outs, axis=0)


# revision 8
# speedup vs baseline: 1.0480x; 1.0480x over previous
"""GCNConv Trainium2 kernel (fallback, measured 2045427ns on HW).

8 NeuronCores, 1D row partition: phase A X@W*d (bf16 out), AllGather bf16,
phase C packed-4 dma_gather (two 1024-idx calls/tile) + DVE mask-select.
"""

import numpy as np

N_NODES = 100000
D_IN = 256
D_OUT = 64
DEG = 16
N_CORES = 8
P = 128
SHARD = N_NODES // N_CORES            # 12500
N_TILES = (SHARD + P - 1) // P        # 98
NPAD = N_TILES * P                    # 12544

_CACHE = {}


def _build_program(n_tiles=N_TILES, deg=DEG, d_in=D_IN, d_out=D_OUT,
                   n_cores=N_CORES):
    import concourse.bacc as bacc
    from concourse import bass, mybir, tile
    from concourse.masks import make_identity

    npad = n_tiles * P
    f32 = mybir.dt.float32
    bf16 = mybir.dt.bfloat16

    nc = bacc.Bacc("TRN2", target_bir_lowering=False, debug=False,
                   num_devices=n_cores)
    XT = nc.dram_tensor("XT", [d_in, npad], f32, kind="ExternalInput").ap()
    W = nc.dram_tensor("W", [d_in, d_out], f32, kind="ExternalInput").ap()
    degs = nc.dram_tensor("degs", [npad, 1], f32, kind="ExternalInput").ap()
    idxB = nc.dram_tensor("idxB", [n_tiles, 2, P, (deg // 2) * P // 16],
                          mybir.dt.int16, kind="ExternalInput").ap()
    msks = nc.dram_tensor("msks", [n_tiles, P, 4 * deg], bf16,
                          kind="ExternalInput").ap()
    out = nc.dram_tensor("out", [npad, d_out], f32, kind="ExternalOutput").ap()

    n_kchunk = d_in // P  # 2

    with tile.TileContext(nc) as tc:
        with (
            tc.tile_pool(name="const", bufs=1) as constp,
            tc.tile_pool(name="xin", bufs=3) as xinp,
            tc.tile_pool(name="xtr", bufs=3) as xtp,
            tc.tile_pool(name="ps", bufs=2, space="PSUM") as psp,
            tc.tile_pool(name="xp", bufs=3) as xpp,
            tc.tile_pool(name="dg", bufs=3) as degp,
            tc.tile_pool(name="ix", bufs=8) as idxp,
            tc.tile_pool(name="gt", bufs=3) as gp,
            tc.tile_pool(name="ot", bufs=3) as outp,
            tc.tile_pool(name="dram", bufs=1, space="DRAM") as dramp,
        ):
            w_sb = constp.tile([P, n_kchunk * d_out], f32)
            for c in range(n_kchunk):
                nc.sync.dma_start(out=w_sb[:, c * d_out:(c + 1) * d_out],
                                  in_=W[c * P:(c + 1) * P, :])

            xpd = dramp.tile([npad, d_out], bf16)
            xfull = dramp.tile([n_cores * npad, d_out], bf16,
                               addr_space="Shared")

            for t in range(n_tiles):
                sl = slice(t * P, (t + 1) * P)
                deg_t = degp.tile([P, 1], f32)
                nc.sync.dma_start(out=deg_t[:], in_=degs[sl, :])
                pso = psp.tile([P, d_out], f32, space="PSUM")
                for c in range(n_kchunk):
                    xT = xtp.tile([P, P], f32, tag="xT")
                    nc.sync.dma_start(out=xT[:],
                                      in_=XT[c * P:(c + 1) * P, sl])
                    nc.tensor.matmul(pso[:], xT[:],
                                     w_sb[:, c * d_out:(c + 1) * d_out],
                                     start=(c == 0), stop=(c == n_kchunk - 1))
                xp_t = xpp.tile([P, d_out], bf16)
                nc.vector.tensor_scalar_mul(xp_t[:], pso[:], deg_t[:, 0:1])
                nc.sync.dma_start(out=xpd[sl, :], in_=xp_t[:])

            nc.gpsimd.collective_compute(
                "AllGather", mybir.AluOpType.bypass,
                replica_groups=[list(range(n_cores))],
                ins=[xpd.opt()], outs=[xfull.opt()],
            )

            xpk = xfull[:].rearrange("(a b) f -> a (b f)", b=4)  # [N/4, 256]
            pk = 4 * d_out
            half = deg // 2
            for t in range(n_tiles):
                sl = slice(t * P, (t + 1) * P)
                msk_t = idxp.tile([P, 4 * deg], bf16, tag="msk")
                nc.sync.dma_start(out=msk_t[:], in_=msks[t])
                deg_c = degp.tile([P, 1], f32, tag="deg_c")
                nc.sync.dma_start(out=deg_c[:], in_=degs[sl, :])
                g_t = gp.tile([P, deg * pk], bf16, tag="g")
                for h in range(2):
                    idx_t = idxp.tile([P, half * P // 16], mybir.dt.int16,
                                      tag="idxb")
                    nc.sync.dma_start(out=idx_t[:], in_=idxB[t, h])
                    nc.gpsimd.dma_gather(
                        g_t[:, h * half * pk:(h + 1) * half * pk]
                        .rearrange("p (s f) -> p s f", s=half),
                        xpk, idx_t[:], half * P, half * P, pk)
                prod = gp.tile([P, deg * pk], bf16, tag="prod")
                nc.vector.tensor_tensor(
                    out=prod[:].rearrange("p (s q f) -> p s q f",
                                          s=deg, q=4),
                    in0=g_t[:].rearrange("p (s q f) -> p s q f",
                                         s=deg, q=4),
                    in1=msk_t[:].rearrange("p (s q) -> p s q", q=4)
                    .to_broadcast([P, deg, 4, d_out]),
                    op=mybir.AluOpType.mult)
                r_t = outp.tile([P, d_out], f32, tag="r")
                nc.vector.tensor_reduce(
                    r_t[:],
                    prod[:].rearrange("p (s q f) -> p f s q", s=deg, q=4),
                    axis=mybir.AxisListType.XY, op=mybir.AluOpType.add)
                o_t = outp.tile([P, d_out], f32, tag="o_t")
                nc.vector.tensor_scalar_mul(o_t[:], r_t[:], deg_c[:, 0:1])
                nc.sync.dma_start(out=out[sl, :], in_=o_t[:])

    nc.compile()
    return nc


def _get_program():
    key = "main"
    if key not in _CACHE:
        _CACHE[key] = _build_program()
    return _CACHE[key]


def _prep_inputs(X, weights, column_index, degrees,
                 n_nodes=N_NODES, n_cores=N_CORES, shard=SHARD, npad=NPAD,
                 deg=DEG):
    import ml_dtypes
    bf16 = ml_dtypes.bfloat16
    X = np.ascontiguousarray(np.asarray(X, dtype=np.float32))
    W = np.ascontiguousarray(np.asarray(weights, dtype=np.float32))
    col = np.asarray(column_index).astype(np.int64, copy=False)
    dg = np.asarray(degrees, dtype=np.float32)

    col32 = (col // shard * npad + col % shard).astype(np.int32)
    col32 = col32.reshape(n_cores, shard, deg)

    n_tiles = npad // 128
    half = deg // 2
    in_maps = []
    pad = npad - shard
    iwB = np.arange(half * 128)
    for c in range(n_cores):
        Xc = np.concatenate(
            [X[c * shard:(c + 1) * shard],
             np.zeros((pad, X.shape[1]), np.float32)], axis=0)
        dgc = np.concatenate(
            [dg[c * shard:(c + 1) * shard],
             np.zeros(pad, np.float32)], axis=0).reshape(npad, 1)
        ixc = np.concatenate(
            [col32[c], np.zeros((pad, deg), np.int32)], axis=0)
        q4, r4 = ixc // 4, ixc % 4
        idxB = np.zeros((n_tiles, 2, 128, half * 128 // 16), np.int16)
        for t in range(n_tiles):
            blk = q4[t * 128:(t + 1) * 128]
            for h in range(2):
                arr = blk[:, h * half:(h + 1) * half].T.reshape(-1)
                wrap = np.zeros((16, half * 128 // 16), np.int16)
                wrap[iwB % 16, iwB // 16] = arr
                idxB[t, h] = np.tile(wrap, (8, 1))
        msk = (r4[:, :, None] == np.arange(4)[None, None, :]).astype(
            np.float32).reshape(n_tiles, 128, deg * 4).astype(bf16)
        XTc = np.ascontiguousarray(Xc.T)
        in_maps.append({"XT": XTc, "W": W, "degs": dgc,
                        "idxB": idxB, "msks": msk})
    return in_maps


def kernel(X, weights, row_pointers, column_index, degrees):
    from concourse.bass_utils import run_bass_kernel_spmd

    rp = np.asarray(row_pointers)
    assert rp.shape[0] == N_NODES + 1
    in_maps = _prep_inputs(X, weights, column_index, degrees)
    nc = _get_program()
    res = run_bass_kernel_spmd(nc, in_maps, core_ids=list(range(N_CORES)))
    outs = [res.results[c]["out"][:SHARD] for c in range(N_CORES)]
    return np.concatenate(outs, axis=0)


# revision 9
# speedup vs baseline: 1.5300x; 1.4600x over previous
"""GCNConv Trainium2 kernel (fallback, measured 2045427ns on HW).

8 NeuronCores, 1D row partition: phase A X@W*d (bf16 out), AllGather bf16,
phase C packed-4 dma_gather (two 1024-idx calls/tile) + DVE mask-select.
"""

import numpy as np

N_NODES = 100000
D_IN = 256
D_OUT = 64
DEG = 16
N_CORES = 8
P = 128
SHARD = N_NODES // N_CORES            # 12500
N_TILES = (SHARD + P - 1) // P        # 98
NPAD = N_TILES * P                    # 12544

_CACHE = {}


def _build_program(n_tiles=N_TILES, deg=DEG, d_in=D_IN, d_out=D_OUT,
                   n_cores=N_CORES):
    import concourse.bacc as bacc
    from concourse import bass, mybir, tile
    from concourse.masks import make_identity

    npad = n_tiles * P
    f32 = mybir.dt.float32
    bf16 = mybir.dt.bfloat16

    nc = bacc.Bacc("TRN2", target_bir_lowering=False, debug=False,
                   num_devices=n_cores, num_swdge_queues=2)
    Xs = nc.dram_tensor("Xs", [npad, d_in], f32, kind="ExternalInput").ap()
    W = nc.dram_tensor("W", [d_in, d_out], f32, kind="ExternalInput").ap()
    degs = nc.dram_tensor("degs", [npad, 1], f32, kind="ExternalInput").ap()
    idxB = nc.dram_tensor("idxB", [n_tiles, 2, P, (deg // 2) * P // 16],
                          mybir.dt.int16, kind="ExternalInput").ap()
    msks = nc.dram_tensor("msks", [n_tiles, P, 4 * deg], bf16,
                          kind="ExternalInput").ap()
    out = nc.dram_tensor("out", [npad, d_out], f32, kind="ExternalOutput").ap()

    n_kchunk = d_in // P  # 2

    with tile.TileContext(nc) as tc:
        with (
            tc.tile_pool(name="const", bufs=1) as constp,
            tc.tile_pool(name="xin", bufs=3) as xinp,
            tc.tile_pool(name="xtr", bufs=3) as xtp,
            tc.tile_pool(name="ps", bufs=2, space="PSUM") as psp,
            tc.tile_pool(name="xp", bufs=3) as xpp,
            tc.tile_pool(name="dg", bufs=3) as degp,
            tc.tile_pool(name="ix", bufs=8) as idxp,
            tc.tile_pool(name="gt", bufs=3) as gp,
            tc.tile_pool(name="ot", bufs=3) as outp,
            tc.tile_pool(name="dram", bufs=1, space="DRAM") as dramp,
        ):
            identity = constp.tile([P, P], f32)
            make_identity(nc, identity[:])
            w_sb = constp.tile([P, n_kchunk * d_out], f32)
            for c in range(n_kchunk):
                nc.sync.dma_start(out=w_sb[:, c * d_out:(c + 1) * d_out],
                                  in_=W[c * P:(c + 1) * P, :])

            xpd = dramp.tile([npad, d_out], bf16)
            xfull = dramp.tile([n_cores * npad, d_out], bf16,
                               addr_space="Shared")

            for t in range(n_tiles):
                sl = slice(t * P, (t + 1) * P)
                xt_t = xinp.tile([P, d_in], f32)
                nc.sync.dma_start(out=xt_t[:], in_=Xs[sl, :])
                deg_t = degp.tile([P, 1], f32)
                nc.sync.dma_start(out=deg_t[:], in_=degs[sl, :])
                pso = psp.tile([P, d_out], f32, space="PSUM")
                for c in range(n_kchunk):
                    psT = psp.tile([P, P], f32, space="PSUM", tag="psT")
                    nc.tensor.transpose(psT[:], xt_t[:, c * P:(c + 1) * P],
                                        identity[:])
                    xT = xtp.tile([P, P], f32, tag="xT")
                    if c % 2 == 0:
                        nc.scalar.copy(xT[:], psT[:])
                    else:
                        nc.vector.tensor_copy(xT[:], psT[:])
                    nc.tensor.matmul(pso[:], xT[:],
                                     w_sb[:, c * d_out:(c + 1) * d_out],
                                     start=(c == 0), stop=(c == n_kchunk - 1))
                xp_t = xpp.tile([P, d_out], bf16)
                nc.vector.tensor_scalar_mul(xp_t[:], pso[:], deg_t[:, 0:1])
                nc.sync.dma_start(out=xpd[sl, :], in_=xp_t[:])

            nc.gpsimd.collective_compute(
                "AllGather", mybir.AluOpType.bypass,
                replica_groups=[list(range(n_cores))],
                ins=[xpd.opt()], outs=[xfull.opt()],
            )

            xpk = xfull[:].rearrange("(a b) f -> a (b f)", b=4)  # [N/4, 256]
            pk = 4 * d_out
            half = deg // 2
            for t in range(n_tiles):
                sl = slice(t * P, (t + 1) * P)
                msk_t = idxp.tile([P, 4 * deg], bf16, tag="msk")
                nc.sync.dma_start(out=msk_t[:], in_=msks[t])
                deg_c = degp.tile([P, 1], f32, tag="deg_c")
                nc.sync.dma_start(out=deg_c[:], in_=degs[sl, :])
                g_t = gp.tile([P, deg * pk], bf16, tag="g")
                for h in range(2):
                    idx_t = idxp.tile([P, half * P // 16], mybir.dt.int16,
                                      tag="idxb")
                    nc.sync.dma_start(out=idx_t[:], in_=idxB[t, h])
                    nc.gpsimd.dma_gather(
                        g_t[:, h * half * pk:(h + 1) * half * pk]
                        .rearrange("p (s f) -> p s f", s=half),
                        xpk, idx_t[:], half * P, half * P, pk,
                        queue_num=h)
                prod = gp.tile([P, deg * pk], bf16, tag="prod")
                nc.vector.tensor_tensor(
                    out=prod[:].rearrange("p (s q f) -> p s q f",
                                          s=deg, q=4),
                    in0=g_t[:].rearrange("p (s q f) -> p s q f",
                                         s=deg, q=4),
                    in1=msk_t[:].rearrange("p (s q) -> p s q", q=4)
                    .to_broadcast([P, deg, 4, d_out]),
                    op=mybir.AluOpType.mult)
                r_t = outp.tile([P, d_out], f32, tag="r")
                nc.vector.tensor_reduce(
                    r_t[:],
                    prod[:].rearrange("p (s q f) -> p f s q", s=deg, q=4),
                    axis=mybir.AxisListType.XY, op=mybir.AluOpType.add)
                o_t = outp.tile([P, d_out], f32, tag="o_t")
                nc.vector.tensor_scalar_mul(o_t[:], r_t[:], deg_c[:, 0:1])
                nc.sync.dma_start(out=out[sl, :], in_=o_t[:])

    nc.compile()
    return nc


def _get_program():
    key = "main"
    if key not in _CACHE:
        _CACHE[key] = _build_program()
    return _CACHE[key]


def _prep_inputs(X, weights, column_index, degrees,
                 n_nodes=N_NODES, n_cores=N_CORES, shard=SHARD, npad=NPAD,
                 deg=DEG):
    import ml_dtypes
    bf16 = ml_dtypes.bfloat16
    X = np.ascontiguousarray(np.asarray(X, dtype=np.float32))
    W = np.ascontiguousarray(np.asarray(weights, dtype=np.float32))
    col = np.asarray(column_index).astype(np.int64, copy=False)
    dg = np.asarray(degrees, dtype=np.float32)

    col32 = (col // shard * npad + col % shard).astype(np.int32)
    col32 = col32.reshape(n_cores, shard, deg)

    n_tiles = npad // 128
    half = deg // 2
    in_maps = []
    pad = npad - shard
    iwB = np.arange(half * 128)
    for c in range(n_cores):
        Xc = np.concatenate(
            [X[c * shard:(c + 1) * shard],
             np.zeros((pad, X.shape[1]), np.float32)], axis=0)
        dgc = np.concatenate(
            [dg[c * shard:(c + 1) * shard],
             np.zeros(pad, np.float32)], axis=0).reshape(npad, 1)
        ixc = np.concatenate(
            [col32[c], np.zeros((pad, deg), np.int32)], axis=0)
        q4, r4 = ixc // 4, ixc % 4
        idxB = np.zeros((n_tiles, 2, 128, half * 128 // 16), np.int16)
        for t in range(n_tiles):
            blk = q4[t * 128:(t + 1) * 128]
            for h in range(2):
                arr = blk[:, h * half:(h + 1) * half].T.reshape(-1)
                wrap = np.zeros((16, half * 128 // 16), np.int16)
                wrap[iwB % 16, iwB // 16] = arr
                idxB[t, h] = np.tile(wrap, (8, 1))
        msk = (r4[:, :, None] == np.arange(4)[None, None, :]).astype(
            np.float32).reshape(n_tiles, 128, deg * 4).astype(bf16)
        in_maps.append({"Xs": Xc, "W": W, "degs": dgc,
                        "idxB": idxB, "msks": msk})
    return in_maps


def kernel(X, weights, row_pointers, column_index, degrees):
    from concourse.bass_utils import run_bass_kernel_spmd

    rp = np.asarray(row_pointers)
    assert rp.shape[0] == N_NODES + 1
    in_maps = _prep_inputs(X, weights, column_index, degrees)
    nc = _get_program()
    res = run_bass_kernel_spmd(nc, in_maps, core_ids=list(range(N_CORES)))
    outs = [res.results[c]["out"][:SHARD] for c in range(N_CORES)]
    return np.concatenate(outs, axis=0)


# revision 10
# speedup vs baseline: 1.5395x; 1.0062x over previous
"""GCNConv Trainium2 kernel (fallback, measured 2045427ns on HW).

8 NeuronCores, 1D row partition: phase A X@W*d (bf16 out), AllGather bf16,
phase C packed-4 dma_gather (two 1024-idx calls/tile) + DVE mask-select.
"""

import numpy as np

N_NODES = 100000
D_IN = 256
D_OUT = 64
DEG = 16
N_CORES = 8
P = 128
SHARD = N_NODES // N_CORES            # 12500
N_TILES = (SHARD + P - 1) // P        # 98
NPAD = N_TILES * P                    # 12544

_CACHE = {}


def _build_program(n_tiles=N_TILES, deg=DEG, d_in=D_IN, d_out=D_OUT,
                   n_cores=N_CORES):
    import concourse.bacc as bacc
    from concourse import bass, mybir, tile
    from concourse.masks import make_identity

    npad = n_tiles * P
    f32 = mybir.dt.float32
    bf16 = mybir.dt.bfloat16

    nc = bacc.Bacc("TRN2", target_bir_lowering=False, debug=False,
                   num_devices=n_cores, num_swdge_queues=4)
    Xs = nc.dram_tensor("Xs", [npad, d_in], f32, kind="ExternalInput").ap()
    W = nc.dram_tensor("W", [d_in, d_out], f32, kind="ExternalInput").ap()
    degs = nc.dram_tensor("degs", [npad, 1], f32, kind="ExternalInput").ap()
    idxB = nc.dram_tensor("idxB", [n_tiles, 2, P, (deg // 2) * P // 16],
                          mybir.dt.int16, kind="ExternalInput").ap()
    msks = nc.dram_tensor("msks", [n_tiles, P, 4 * deg], bf16,
                          kind="ExternalInput").ap()
    out = nc.dram_tensor("out", [npad, d_out], f32, kind="ExternalOutput").ap()

    n_kchunk = d_in // P  # 2

    with tile.TileContext(nc) as tc:
        with (
            tc.tile_pool(name="const", bufs=1) as constp,
            tc.tile_pool(name="xin", bufs=3) as xinp,
            tc.tile_pool(name="xtr", bufs=3) as xtp,
            tc.tile_pool(name="ps", bufs=2, space="PSUM") as psp,
            tc.tile_pool(name="xp", bufs=3) as xpp,
            tc.tile_pool(name="dg", bufs=3) as degp,
            tc.tile_pool(name="ix", bufs=8) as idxp,
            tc.tile_pool(name="gt", bufs=3) as gp,
            tc.tile_pool(name="ot", bufs=3) as outp,
            tc.tile_pool(name="dram", bufs=1, space="DRAM") as dramp,
        ):
            identity = constp.tile([P, P], f32)
            make_identity(nc, identity[:])
            w_sb = constp.tile([P, n_kchunk * d_out], f32)
            for c in range(n_kchunk):
                nc.sync.dma_start(out=w_sb[:, c * d_out:(c + 1) * d_out],
                                  in_=W[c * P:(c + 1) * P, :])

            xpd = dramp.tile([npad, d_out], bf16)
            xfull = dramp.tile([n_cores * npad, d_out], bf16,
                               addr_space="Shared")

            for t in range(n_tiles):
                sl = slice(t * P, (t + 1) * P)
                xt_t = xinp.tile([P, d_in], f32)
                nc.sync.dma_start(out=xt_t[:], in_=Xs[sl, :])
                deg_t = degp.tile([P, 1], f32)
                nc.sync.dma_start(out=deg_t[:], in_=degs[sl, :])
                pso = psp.tile([P, d_out], f32, space="PSUM")
                for c in range(n_kchunk):
                    psT = psp.tile([P, P], f32, space="PSUM", tag="psT")
                    nc.tensor.transpose(psT[:], xt_t[:, c * P:(c + 1) * P],
                                        identity[:])
                    xT = xtp.tile([P, P], f32, tag="xT")
                    if c % 2 == 0:
                        nc.scalar.copy(xT[:], psT[:])
                    else:
                        nc.vector.tensor_copy(xT[:], psT[:])
                    nc.tensor.matmul(pso[:], xT[:],
                                     w_sb[:, c * d_out:(c + 1) * d_out],
                                     start=(c == 0), stop=(c == n_kchunk - 1))
                xp_t = xpp.tile([P, d_out], bf16)
                nc.vector.tensor_scalar_mul(xp_t[:], pso[:], deg_t[:, 0:1])
                nc.sync.dma_start(out=xpd[sl, :], in_=xp_t[:])

            nc.gpsimd.collective_compute(
                "AllGather", mybir.AluOpType.bypass,
                replica_groups=[list(range(n_cores))],
                ins=[xpd.opt()], outs=[xfull.opt()],
            )

            xpk = xfull[:].rearrange("(a b) f -> a (b f)", b=4)  # [N/4, 256]
            pk = 4 * d_out
            half = deg // 2
            for t in range(n_tiles):
                sl = slice(t * P, (t + 1) * P)
                msk_t = idxp.tile([P, 4 * deg], bf16, tag="msk")
                nc.sync.dma_start(out=msk_t[:], in_=msks[t])
                deg_c = degp.tile([P, 1], f32, tag="deg_c")
                nc.sync.dma_start(out=deg_c[:], in_=degs[sl, :])
                g_t = gp.tile([P, deg * pk], bf16, tag="g")
                for h in range(2):
                    idx_t = idxp.tile([P, half * P // 16], mybir.dt.int16,
                                      tag="idxb")
                    nc.sync.dma_start(out=idx_t[:], in_=idxB[t, h])
                    nc.gpsimd.dma_gather(
                        g_t[:, h * half * pk:(h + 1) * half * pk]
                        .rearrange("p (s f) -> p s f", s=half),
                        xpk, idx_t[:], half * P, half * P, pk,
                        queue_num=(2 * t + h) % 4)
                prod = gp.tile([P, deg * pk], bf16, tag="prod")
                nc.vector.tensor_tensor(
                    out=prod[:].rearrange("p (s q f) -> p s q f",
                                          s=deg, q=4),
                    in0=g_t[:].rearrange("p (s q f) -> p s q f",
                                         s=deg, q=4),
                    in1=msk_t[:].rearrange("p (s q) -> p s q", q=4)
                    .to_broadcast([P, deg, 4, d_out]),
                    op=mybir.AluOpType.mult)
                r_t = outp.tile([P, d_out], f32, tag="r")
                nc.vector.tensor_reduce(
                    r_t[:],
                    prod[:].rearrange("p (s q f) -> p f s q", s=deg, q=4),
                    axis=mybir.AxisListType.XY, op=mybir.AluOpType.add)
                o_t = outp.tile([P, d_out], f32, tag="o_t")
                nc.vector.tensor_scalar_mul(o_t[:], r_t[:], deg_c[:, 0:1])
                nc.sync.dma_start(out=out[sl, :], in_=o_t[:])

    nc.compile()
    return nc


def _get_program():
    key = "main"
    if key not in _CACHE:
        _CACHE[key] = _build_program()
    return _CACHE[key]


def _prep_inputs(X, weights, column_index, degrees,
                 n_nodes=N_NODES, n_cores=N_CORES, shard=SHARD, npad=NPAD,
                 deg=DEG):
    import ml_dtypes
    bf16 = ml_dtypes.bfloat16
    X = np.ascontiguousarray(np.asarray(X, dtype=np.float32))
    W = np.ascontiguousarray(np.asarray(weights, dtype=np.float32))
    col = np.asarray(column_index).astype(np.int64, copy=False)
    dg = np.asarray(degrees, dtype=np.float32)

    col32 = (col // shard * npad + col % shard).astype(np.int32)
    col32 = col32.reshape(n_cores, shard, deg)

    n_tiles = npad // 128
    half = deg // 2
    in_maps = []
    pad = npad - shard
    iwB = np.arange(half * 128)
    for c in range(n_cores):
        Xc = np.concatenate(
            [X[c * shard:(c + 1) * shard],
             np.zeros((pad, X.shape[1]), np.float32)], axis=0)
        dgc = np.concatenate(
            [dg[c * shard:(c + 1) * shard],
             np.zeros(pad, np.float32)], axis=0).reshape(npad, 1)
        ixc = np.concatenate(
            [col32[c], np.zeros((pad, deg), np.int32)], axis=0)
        q4, r4 = ixc // 4, ixc % 4
        idxB = np.zeros((n_tiles, 2, 128, half * 128 // 16), np.int16)
        for t in range(n_tiles):
            blk = q4[t * 128:(t + 1) * 128]
            for h in range(2):
                arr = blk[:, h * half:(h + 1) * half].T.reshape(-1)
                wrap = np.zeros((16, half * 128 // 16), np.int16)
                wrap[iwB % 16, iwB // 16] = arr
                idxB[t, h] = np.tile(wrap, (8, 1))
        msk = (r4[:, :, None] == np.arange(4)[None, None, :]).astype(
            np.float32).reshape(n_tiles, 128, deg * 4).astype(bf16)
        in_maps.append({"Xs": Xc, "W": W, "degs": dgc,
                        "idxB": idxB, "msks": msk})
    return in_maps


def kernel(X, weights, row_pointers, column_index, degrees):
    from concourse.bass_utils import run_bass_kernel_spmd

    rp = np.asarray(row_pointers)
    assert rp.shape[0] == N_NODES + 1
    in_maps = _prep_inputs(X, weights, column_index, degrees)
    nc = _get_program()
    res = run_bass_kernel_spmd(nc, in_maps, core_ids=list(range(N_CORES)))
    outs = [res.results[c]["out"][:SHARD] for c in range(N_CORES)]
    return np.concatenate(outs, axis=0)


# revision 11
# speedup vs baseline: 2.0970x; 1.3621x over previous
"""GCNConv Trainium2 kernel (fallback, measured 2045427ns on HW).

8 NeuronCores, 1D row partition: phase A X@W*d (bf16 out), AllGather bf16,
phase C packed-4 dma_gather (two 1024-idx calls/tile) + DVE mask-select.
"""

import numpy as np

N_NODES = 100000
D_IN = 256
D_OUT = 64
DEG = 16
N_CORES = 8
P = 128
SHARD = N_NODES // N_CORES            # 12500
N_TILES = (SHARD + P - 1) // P        # 98
NPAD = N_TILES * P                    # 12544

_CACHE = {}


def _build_program(n_tiles=N_TILES, deg=DEG, d_in=D_IN, d_out=D_OUT,
                   n_cores=N_CORES):
    import concourse.bacc as bacc
    from concourse import bass, mybir, tile
    from concourse.masks import make_identity

    npad = n_tiles * P
    f32 = mybir.dt.float32
    bf16 = mybir.dt.bfloat16

    nc = bacc.Bacc("TRN2", target_bir_lowering=False, debug=False,
                   num_devices=n_cores, num_swdge_queues=4)
    Xs = nc.dram_tensor("Xs", [npad, d_in], f32, kind="ExternalInput").ap()
    W = nc.dram_tensor("W", [d_in, d_out], f32, kind="ExternalInput").ap()
    degs = nc.dram_tensor("degs", [npad, 1], f32, kind="ExternalInput").ap()
    idxB = nc.dram_tensor("idxB", [n_tiles, 2, P, (deg // 2) * P // 16],
                          mybir.dt.int16, kind="ExternalInput").ap()
    msks = nc.dram_tensor("msks", [n_tiles, P, 4 * deg], bf16,
                          kind="ExternalInput").ap()
    out = nc.dram_tensor("out", [npad, d_out], f32, kind="ExternalOutput").ap()

    n_kchunk = d_in // P  # 2

    with tile.TileContext(nc) as tc:
        with (
            tc.tile_pool(name="const", bufs=1) as constp,
            tc.tile_pool(name="xin", bufs=3) as xinp,
            tc.tile_pool(name="xtr", bufs=3) as xtp,
            tc.tile_pool(name="ps", bufs=2, space="PSUM") as psp,
            tc.tile_pool(name="xp", bufs=3) as xpp,
            tc.tile_pool(name="dg", bufs=3) as degp,
            tc.tile_pool(name="ix", bufs=8) as idxp,
            tc.tile_pool(name="gt", bufs=3) as gp,
            tc.tile_pool(name="ot", bufs=3) as outp,
            tc.tile_pool(name="dram", bufs=1, space="DRAM") as dramp,
        ):
            identity = constp.tile([P, P], f32)
            make_identity(nc, identity[:])
            w_sb = constp.tile([P, n_kchunk * d_out], f32)
            for c in range(n_kchunk):
                nc.sync.dma_start(out=w_sb[:, c * d_out:(c + 1) * d_out],
                                  in_=W[c * P:(c + 1) * P, :])

            xpd = dramp.tile([npad, d_out], bf16)
            xfull = dramp.tile([n_cores * npad, d_out], bf16,
                               addr_space="Shared")

            for t in range(n_tiles):
                sl = slice(t * P, (t + 1) * P)
                xt_t = xinp.tile([P, d_in], f32)
                nc.sync.dma_start(out=xt_t[:], in_=Xs[sl, :])
                deg_t = degp.tile([P, 1], f32)
                nc.sync.dma_start(out=deg_t[:], in_=degs[sl, :])
                pso = psp.tile([P, d_out], f32, space="PSUM")
                for c in range(n_kchunk):
                    psT = psp.tile([P, P], f32, space="PSUM", tag="psT")
                    nc.tensor.transpose(psT[:], xt_t[:, c * P:(c + 1) * P],
                                        identity[:])
                    xT = xtp.tile([P, P], f32, tag="xT")
                    if c % 2 == 0:
                        nc.scalar.copy(xT[:], psT[:])
                    else:
                        nc.vector.tensor_copy(xT[:], psT[:])
                    nc.tensor.matmul(pso[:], xT[:],
                                     w_sb[:, c * d_out:(c + 1) * d_out],
                                     start=(c == 0), stop=(c == n_kchunk - 1))
                xp_t = xpp.tile([P, d_out], bf16)
                nc.vector.tensor_scalar_mul(xp_t[:], pso[:], deg_t[:, 0:1])
                nc.sync.dma_start(out=xpd[sl, :], in_=xp_t[:])

            nc.gpsimd.collective_compute(
                "AllGather", mybir.AluOpType.bypass,
                replica_groups=[list(range(n_cores))],
                ins=[xpd.opt()], outs=[xfull.opt()],
            )

            xpk = xfull[:].rearrange("(a b) f -> a (b f)", b=4)  # [N/4, 256]
            pk = 4 * d_out
            half = deg // 2
            for t in range(n_tiles):
                sl = slice(t * P, (t + 1) * P)
                msk_t = idxp.tile([P, 4 * deg], bf16, tag="msk")
                nc.sync.dma_start(out=msk_t[:], in_=msks[t])
                deg_c = degp.tile([P, 1], f32, tag="deg_c")
                nc.sync.dma_start(out=deg_c[:], in_=degs[sl, :])
                g_t = gp.tile([P, deg * pk], bf16, tag="g")
                qtr = deg // 4
                for h in range(2):
                    idx_t = idxp.tile([P, half * P // 16], mybir.dt.int16,
                                      tag="idxb")
                    nc.sync.dma_start(out=idx_t[:], in_=idxB[t, h])
                    for q in range(2):
                        sb = h * half + q * qtr
                        nc.gpsimd.dma_gather(
                            g_t[:, sb * pk:(sb + qtr) * pk]
                            .rearrange("p (s f) -> p s f", s=qtr),
                            xpk,
                            idx_t[:, q * qtr * P // 16:(q + 1) * qtr * P // 16],
                            qtr * P, qtr * P, pk,
                            queue_num=2 * h + q)
                prod = gp.tile([P, deg * pk], bf16, tag="prod")
                nc.vector.tensor_tensor(
                    out=prod[:].rearrange("p (s q f) -> p s q f",
                                          s=deg, q=4),
                    in0=g_t[:].rearrange("p (s q f) -> p s q f",
                                         s=deg, q=4),
                    in1=msk_t[:].rearrange("p (s q) -> p s q", q=4)
                    .to_broadcast([P, deg, 4, d_out]),
                    op=mybir.AluOpType.mult)
                # dense bf16 tree-reduce over (s, q): 4096 -> 64
                tr = gp.tile([P, deg * pk // 2], bf16, tag="tr")
                nc.vector.tensor_add(tr[:], prod[:, :deg * pk // 2],
                                     prod[:, deg * pk // 2:])
                w_cur = deg * pk // 2
                while w_cur > d_out:
                    nc.vector.tensor_add(tr[:, :w_cur // 2],
                                         tr[:, :w_cur // 2],
                                         tr[:, w_cur // 2:w_cur])
                    w_cur //= 2
                o_t = outp.tile([P, d_out], f32, tag="o_t")
                nc.vector.tensor_scalar_mul(o_t[:], tr[:, :d_out],
                                            deg_c[:, 0:1])
                nc.sync.dma_start(out=out[sl, :], in_=o_t[:])

    nc.compile()
    return nc


def _get_program():
    key = "main"
    if key not in _CACHE:
        _CACHE[key] = _build_program()
    return _CACHE[key]


def _prep_inputs(X, weights, column_index, degrees,
                 n_nodes=N_NODES, n_cores=N_CORES, shard=SHARD, npad=NPAD,
                 deg=DEG):
    import ml_dtypes
    bf16 = ml_dtypes.bfloat16
    X = np.ascontiguousarray(np.asarray(X, dtype=np.float32))
    W = np.ascontiguousarray(np.asarray(weights, dtype=np.float32))
    col = np.asarray(column_index).astype(np.int64, copy=False)
    dg = np.asarray(degrees, dtype=np.float32)

    col32 = (col // shard * npad + col % shard).astype(np.int32)
    col32 = col32.reshape(n_cores, shard, deg)

    n_tiles = npad // 128
    half = deg // 2
    in_maps = []
    pad = npad - shard
    iwB = np.arange(half * 128)
    for c in range(n_cores):
        Xc = np.concatenate(
            [X[c * shard:(c + 1) * shard],
             np.zeros((pad, X.shape[1]), np.float32)], axis=0)
        dgc = np.concatenate(
            [dg[c * shard:(c + 1) * shard],
             np.zeros(pad, np.float32)], axis=0).reshape(npad, 1)
        ixc = np.concatenate(
            [col32[c], np.zeros((pad, deg), np.int32)], axis=0)
        q4, r4 = ixc // 4, ixc % 4
        idxB = np.zeros((n_tiles, 2, 128, half * 128 // 16), np.int16)
        for t in range(n_tiles):
            blk = q4[t * 128:(t + 1) * 128]
            for h in range(2):
                arr = blk[:, h * half:(h + 1) * half].T.reshape(-1)
                wrap = np.zeros((16, half * 128 // 16), np.int16)
                wrap[iwB % 16, iwB // 16] = arr
                idxB[t, h] = np.tile(wrap, (8, 1))
        msk = (r4[:, :, None] == np.arange(4)[None, None, :]).astype(
            np.float32).reshape(n_tiles, 128, deg * 4).astype(bf16)
        in_maps.append({"Xs": Xc, "W": W, "degs": dgc,
                        "idxB": idxB, "msks": msk})
    return in_maps


def kernel(X, weights, row_pointers, column_index, degrees):
    from concourse.bass_utils import run_bass_kernel_spmd

    rp = np.asarray(row_pointers)
    assert rp.shape[0] == N_NODES + 1
    in_maps = _prep_inputs(X, weights, column_index, degrees)
    nc = _get_program()
    res = run_bass_kernel_spmd(nc, in_maps, core_ids=list(range(N_CORES)))
    outs = [res.results[c]["out"][:SHARD] for c in range(N_CORES)]
    return np.concatenate(outs, axis=0)
